# revision 8
# baseline (speedup 1.0000x reference)
"""Trainium2 Bass kernel: 3x3 stride-1 pad-1 Conv2D, NCHW.

Problem: x (32,128,56,56) f32, weight (256,128,3,3) OIHW, bias (256,)
-> out (32,256,56,56) f32.

Strategy: data-parallel over batch N across 8 NeuronCores (4 images per
core), weights/bias replicated. Per core: implicit GEMM — C_in=128 is
exactly the SBUF partition dim; for each of the 9 filter taps we issue a
128x128 (ci x co-chunk) matmul against a shifted window of the
host-padded image, accumulating all 9 taps into one PSUM bank. f32r
matmul (1 cycle/row for free dim >= 256) gives ~4x over plain fp32.
"""

import numpy as np

import concourse.bass as bass
import concourse.mybir as mybir
import concourse.tile as tile
from concourse import bacc
from concourse.bass_utils import run_bass_kernel_spmd

N_CORES = 8
N_FULL = 32
N_PER_CORE = N_FULL // N_CORES  # 4
CIN = 128
COUT = 256
H = W = 56
HP = WP = 58  # padded spatial
R = 8  # output rows per matmul tile
NT = H // R  # 7 row-tiles per image
NFREE = R * W  # 448 (<= 512 fp32 PSUM bank limit)
F32 = mybir.dt.float32
F32R = mybir.dt.float32r

# Module-level knobs for the dev harness (test.py). The grading harness
# just calls kernel(**inputs) and gets the default (no-trace) path.
TRACE = False
LAST_RESULT = None

_prog = None


def _build_program():
    nc = bacc.Bacc("TRN2", target_bir_lowering=False, debug=False)
    x_d = nc.declare_dram_parameter("x", [N_PER_CORE, CIN, HP * WP], F32R, isOutput=False)
    w_d = nc.declare_dram_parameter("wt", [CIN, 9 * COUT], F32R, isOutput=False)
    b_d = nc.declare_dram_parameter("bias", [COUT], F32, isOutput=False)
    out_d = nc.declare_dram_parameter(
        "out", [N_PER_CORE, 2, 128, H * W], F32, isOutput=True
    )

    with tile.TileContext(nc) as tc:
        with (
            tc.tile_pool(name="const", bufs=1) as const_pool,
            tc.tile_pool(name="xin", bufs=2) as x_pool,
            tc.tile_pool(name="outp", bufs=4) as out_pool,
            tc.tile_pool(name="psum", bufs=6, space="PSUM") as psum_pool,
        ):
            w_sb = const_pool.tile([CIN, 9 * COUT], F32R)
            nc.sync.dma_start(out=w_sb[:], in_=w_d[:])
            bias_sb = const_pool.tile([128, 2], F32)
            for c in range(2):
                nc.sync.dma_start(
                    out=bias_sb[:, c : c + 1],
                    in_=b_d[c * 128 : (c + 1) * 128].rearrange("(p one) -> p one", one=1),
                )

            for i in range(N_PER_CORE):
                x_t = x_pool.tile([CIN, HP * WP], F32R)
                nc.sync.dma_start(out=x_t[:], in_=x_d[i])
                x_img = x_t[:].rearrange("p (h w) -> p h w", w=WP)

                for c in range(2):
                    for r in range(NT):
                        psum_t = psum_pool.tile([128, NFREE], F32)
                        psum_v = psum_t[:].rearrange("p (r w) -> p r w", w=W)
                        for k in range(9):
                            kh, kw = divmod(k, 3)
                            rhs = x_img[:, r * R + kh : r * R + kh + R, kw : kw + W]
                            lhsT = w_sb[:, k * COUT + c * 128 : k * COUT + c * 128 + 128]
                            nc.tensor.matmul(
                                psum_v,
                                lhsT=lhsT,
                                rhs=rhs,
                                start=(k == 0),
                                stop=(k == 8),
                            )
                        out_t = out_pool.tile([128, NFREE], F32)
                        nc.vector.tensor_scalar_add(
                            out_t[:], psum_t[:], bias_sb[:, c : c + 1]
                        )
                        nc.sync.dma_start(
                            out=out_d[i, c][:, r * NFREE : (r + 1) * NFREE],
                            in_=out_t[:],
                        )
    nc.compile()
    return nc


def kernel(x: np.ndarray, weight: np.ndarray, bias: np.ndarray) -> np.ndarray:
    global _prog, LAST_RESULT
    x = np.ascontiguousarray(x, dtype=np.float32)
    weight = np.ascontiguousarray(weight, dtype=np.float32)
    bias = np.ascontiguousarray(bias, dtype=np.float32)

    # Host-side prep: pad spatial dims, shard batch, pre-transpose weights.
    x_pad = np.zeros((N_FULL, CIN, HP, WP), dtype=np.float32)
    x_pad[:, :, 1:-1, 1:-1] = x
    x_pad = x_pad.reshape(N_FULL, CIN, HP * WP)

    # wt[ci, k*256 + co] = weight[co, ci, kh, kw], k = kh*3+kw
    wt = np.ascontiguousarray(
        weight.transpose(1, 2, 3, 0).reshape(CIN, 9 * COUT)
    )

    if _prog is None:
        _prog = _build_program()

    in_maps = [
        {
            "x": np.ascontiguousarray(x_pad[i * N_PER_CORE : (i + 1) * N_PER_CORE]),
            "wt": wt,
            "bias": bias,
        }
        for i in range(N_CORES)
    ]
    res = run_bass_kernel_spmd(_prog, in_maps, list(range(N_CORES)), trace=TRACE)
    LAST_RESULT = res
    out = np.concatenate([r["out"] for r in res.results], axis=0)
    return out.reshape(N_FULL, COUT, H, W)


# revision 9
# speedup vs baseline: 1.0342x; 1.0342x over previous
"""Trainium2 Bass kernel: 3x3 stride-1 pad-1 Conv2D, NCHW.

Problem: x (32,128,56,56) f32, weight (256,128,3,3) OIHW, bias (256,)
-> out (32,256,56,56) f32.

Strategy: data-parallel over batch N across 8 NeuronCores (4 images per
core), weights/bias replicated. Per core: implicit GEMM — C_in=128 is
exactly the SBUF partition dim; for each of the 9 filter taps we issue a
128x128 (ci x co-chunk) matmul against a shifted window of the
host-padded image, accumulating all 9 taps into one PSUM bank. f32r
matmul (1 cycle/row for free dim >= 256) gives ~4x over plain fp32.
"""

import numpy as np

import concourse.bass as bass
import concourse.mybir as mybir
import concourse.tile as tile
from concourse import bacc
from concourse.bass_utils import run_bass_kernel_spmd

N_CORES = 8
N_FULL = 32
N_PER_CORE = N_FULL // N_CORES  # 4
CIN = 128
COUT = 256
H = W = 56
HP = WP = 58  # padded spatial
R = 8  # output rows per matmul tile
NT = H // R  # 7 row-tiles per image
NFREE = R * W  # 448 (<= 512 fp32 PSUM bank limit)
F32 = mybir.dt.float32
F32R = mybir.dt.float32r

# Module-level knobs for the dev harness (test.py). The grading harness
# just calls kernel(**inputs) and gets the default (no-trace) path.
TRACE = False
LAST_RESULT = None

_prog = None


def _build_program():
    nc = bacc.Bacc("TRN2", target_bir_lowering=False, debug=False)
    x_d = nc.declare_dram_parameter("x", [N_PER_CORE, CIN, HP * WP], F32R, isOutput=False)
    w_d = nc.declare_dram_parameter("wt", [CIN, 9 * COUT], F32R, isOutput=False)
    b_d = nc.declare_dram_parameter("bias", [COUT], F32, isOutput=False)
    out_d = nc.declare_dram_parameter(
        "out", [N_PER_CORE, 2, 128, H * W], F32, isOutput=True
    )

    CH = (R + 2) * WP  # one chunk: R output rows + 2 halo rows of padded input

    with tile.TileContext(nc) as tc:
        with (
            tc.tile_pool(name="const", bufs=1) as const_pool,
            tc.tile_pool(name="xin", bufs=8) as x_pool,
            tc.tile_pool(name="outp", bufs=4) as out_pool,
            tc.tile_pool(name="psum", bufs=6, space="PSUM") as psum_pool,
        ):
            bias_sb = const_pool.tile([128, 2], F32)
            for c in range(2):
                nc.sync.dma_start(
                    out=bias_sb[:, c : c + 1],
                    in_=b_d[c * 128 : (c + 1) * 128].rearrange("(p one) -> p one", one=1),
                )
            w_sb = const_pool.tile([CIN, 9 * COUT], F32R)
            nc.sync.dma_start(out=w_sb[:], in_=w_d[:])

            # Per-image, per-row-block input chunks (overlapping halo rows) so
            # the first matmuls only wait on a ~300KB DMA, not whole images.
            x_view = x_d[:].rearrange("n p (h w) -> n p h w", w=WP)
            x_tiles = {}

            def load_chunk(i, r):
                x_c = x_pool.tile([CIN, CH], F32R)
                nc.sync.dma_start(
                    out=x_c[:],
                    in_=x_view[i][:, r * R : r * R + R + 2, :],
                )
                x_tiles[(i, r)] = x_c

            def compute_tile(i, c, r):
                x_img = x_tiles[(i, r)][:].rearrange("p (h w) -> p h w", w=WP)
                psum_t = psum_pool.tile([128, NFREE], F32)
                psum_v = psum_t[:].rearrange("p (r w) -> p r w", w=W)
                for k in range(9):
                    kh, kw = divmod(k, 3)
                    rhs = x_img[:, kh : kh + R, kw : kw + W]
                    lhsT = w_sb[:, k * COUT + c * 128 : k * COUT + c * 128 + 128]
                    nc.tensor.matmul(
                        psum_v, lhsT=lhsT, rhs=rhs, start=(k == 0), stop=(k == 8)
                    )
                out_t = out_pool.tile([128, NFREE], F32)
                nc.vector.tensor_scalar_add(out_t[:], psum_t[:], bias_sb[:, c : c + 1])
                nc.sync.dma_start(
                    out=out_d[i, c][:, r * NFREE : (r + 1) * NFREE], in_=out_t[:]
                )

            # Emission order = DMA queue order: first two chunks land before
            # compute starts; each chunk is consumed by both co-chunks, then
            # its pool slot recycles.
            load_chunk(0, 0)
            load_chunk(0, 1)
            for i in range(N_PER_CORE):
                for r in range(NT):
                    nxt = (i, r + 2) if r + 2 < NT else (i + 1, (r + 2) % NT)
                    if nxt[0] < N_PER_CORE and nxt not in x_tiles:
                        load_chunk(*nxt)
                    for c in range(2):
                        compute_tile(i, c, r)
                    del x_tiles[(i, r)]
    nc.compile()
    return nc


def kernel(x: np.ndarray, weight: np.ndarray, bias: np.ndarray) -> np.ndarray:
    global _prog, LAST_RESULT
    x = np.ascontiguousarray(x, dtype=np.float32)
    weight = np.ascontiguousarray(weight, dtype=np.float32)
    bias = np.ascontiguousarray(bias, dtype=np.float32)

    # Host-side prep: pad spatial dims, shard batch, pre-transpose weights.
    x_pad = np.zeros((N_FULL, CIN, HP, WP), dtype=np.float32)
    x_pad[:, :, 1:-1, 1:-1] = x
    x_pad = x_pad.reshape(N_FULL, CIN, HP * WP)

    # wt[ci, k*256 + co] = weight[co, ci, kh, kw], k = kh*3+kw
    wt = np.ascontiguousarray(
        weight.transpose(1, 2, 3, 0).reshape(CIN, 9 * COUT)
    )

    if _prog is None:
        _prog = _build_program()

    in_maps = [
        {
            "x": np.ascontiguousarray(x_pad[i * N_PER_CORE : (i + 1) * N_PER_CORE]),
            "wt": wt,
            "bias": bias,
        }
        for i in range(N_CORES)
    ]
    res = run_bass_kernel_spmd(_prog, in_maps, list(range(N_CORES)), trace=TRACE)
    LAST_RESULT = res
    out = np.concatenate([r["out"] for r in res.results], axis=0)
    return out.reshape(N_FULL, COUT, H, W)
